# revision 1
# baseline (speedup 1.0000x reference)
"""BWGNN_Hetero Trainium2 kernel (8 NeuronCores, SPMD).

Math restructure of the reference:
  - poly_conv with THETAS (Bernstein, D=2) shares the Krylov basis
    P0 = h, P1 = L~ P0, P2 = L~ P1 across all three thetas, where
    L~ x = x - dinv * segsum((x*dinv)[src], dst).
  - concat(h_theta) @ W3 collapses to sum_k P_k @ V_k with
    V_k = sum_i THETA[i][k] * W3[i*H:(i+1)*H] (host-precomputed).
  - out = leaky(sum_r sum_k P_k^r @ V_k + 2*b3)

Sharding: nodes split 8 ways (6250/core, padded to 6272 = 49*128 blocks).
Edges live on their dst-owner core, grouped per 128-node dst block and
split by src table half (dma_gather indices are int16; the all-gathered
feature table has 50176 rows, so it is addressed as two 25088-row halves).
Aggregation = gathered-rows (lhsT) x one-hot(dstoff) (rhs) matmuls
accumulated in PSUM, producing feature-major agg^T [64,128] per block.
Between hops, scaled features are all-gathered (1.6MB/core).
"""

import math
import os
import sys

import ml_dtypes
import numpy as np

for _p in ("/opt/trn_rl_repo", "/root/.axon_site/_ro/trn_rl_repo"):
    if os.path.isdir(_p) and _p not in sys.path:
        sys.path.insert(0, _p)

import concourse.bacc as bacc
import concourse.bass as bass
import concourse.mybir as mybir
import concourse.tile as tile
from concourse.bass_utils import run_bass_kernel_spmd

F32 = mybir.dt.float32
BF16 = mybir.dt.bfloat16
I16 = mybir.dt.int16
AF = mybir.ActivationFunctionType


class Cfg:
    def __init__(self, N=50000, F=128, H=64, NCORES=8, BPC=None):
        self.N = N
        self.F = F
        self.H = H
        self.NCORES = NCORES
        self.NPC = N // NCORES              # real nodes per core
        self.BLK = 64                       # dst nodes per aggregation block
        self.NBLK = (self.NPC + self.BLK - 1) // self.BLK
        self.NPAD = self.NBLK * self.BLK    # padded nodes per core
        assert self.NPAD % 128 == 0
        self.NTAB = self.NPAD * NCORES      # padded global table rows
        self.HALF = self.NTAB // 2          # rows per gather-table half
        assert self.HALF <= 32768, "int16 gather index range"
        if BPC is None:
            BPC = next(b for b in (7, 5, 4, 3, 2, 1) if self.NBLK % b == 0)
        self.BPC = BPC                      # blocks per gather call
        assert self.NBLK % BPC == 0
        self.NCALL = self.NBLK // BPC


THETAS = None


def _calc_thetas(d=2):
    thetas = []
    for i in range(d + 1):
        p1 = np.zeros(i + 1)
        p1[i] = 0.5 ** i
        m = d - i
        p2 = np.array([math.comb(m, j) * (-0.5) ** j for j in range(m + 1)])
        c = np.convolve(p1, p2)
        beta = math.factorial(i) * math.factorial(d - i) / math.factorial(d + 1)
        thetas.append(c / beta)
    return np.stack(thetas)  # [3, 3] increasing power


THETAS = _calc_thetas(2)


def _pack_idx16(flat, cfg):
    """Q7 layout: idx i -> partition i%16, free i//16; replicated x8 groups."""
    n = len(flat)
    m16 = n // 16
    arr = flat.astype(np.int16).reshape(m16, 16).T  # [16, m16]
    return np.tile(arr, (8, 1))  # [128, m16]


def _greedy_balance(cnt4, cfg):
    """Assign nodes to BLK-sized blocks minimizing max per-(rel,half) count.

    cnt4: [NPC, 4] in-degree by (rel, half). Returns local padded position
    per node. Shared by both relations (the output layout must match).
    """
    c = cfg
    order = np.argsort(-cnt4.sum(1), kind="stable")
    blk_cnt = np.zeros((c.NBLK, 4), np.int64)
    blk_n = np.zeros(c.NBLK, np.int64)
    pos = np.zeros(c.NPC, np.int64)
    BIG = 1 << 40
    for n in order:
        score = (blk_cnt + cnt4[n]).max(axis=1) + blk_cnt.max(axis=1)
        score[blk_n >= c.BLK] = BIG
        b = int(np.argmin(score))
        pos[n] = b * c.BLK + blk_n[b]
        blk_n[b] += 1
        blk_cnt[b] += cnt4[n]
    return pos


def _prep_relation(feat, src, dst, cfg, POS):
    """Per-relation host preprocessing (vectorized, node-permuted)."""
    c = cfg
    deg = np.bincount(dst, minlength=c.N).astype(np.float32)
    dinv = np.clip(deg, 1.0, None) ** -0.5

    src_pad = POS[src]
    owner = dst // c.NPC
    d_pos = POS[dst]
    all_slots = []
    lh = 0
    for core in range(c.NCORES):
        m = owner == core
        s_p = src_pad[m]
        d_loc = d_pos[m] - core * c.NPAD
        blk = d_loc // c.BLK
        off = (d_loc % c.BLK).astype(np.float32)
        half = (s_p >= c.HALF).astype(np.int64)
        idxv = s_p - half * c.HALF
        key = blk * 2 + half
        order = np.lexsort((idxv, key))
        key_s, idx_s, off_s = key[order], idxv[order], off[order]
        counts = np.bincount(key_s, minlength=c.NBLK * 2)
        starts = np.concatenate(([0], np.cumsum(counts)[:-1]))
        within = np.arange(len(key_s)) - starts[key_s]
        lh = max(lh, int(counts.max()) if len(counts) else 0)
        all_slots.append((key_s, within, idx_s, off_s))
    return all_slots, dinv, lh


def _finalize_relation(all_slots, dinv, feat, cfg, LH, POS):
    c = cfg
    NS = -(-LH // 128)
    LHL = NS * 128                      # sentinel-padded layout length
    per_core = []
    for core in range(c.NCORES):
        key_s, within, idx_s, off_s = all_slots[core]
        idx_slots = np.zeros((c.NBLK * 2, LH), np.int64)
        off_slots = np.full((c.NBLK * 2, LHL), 255.0, np.float32)
        idx_slots[key_s, within] = idx_s
        off_slots[key_s, within] = off_s

        # idx_packed: one call per (blk, half), LH idxs each; pad slots
        # gather row 0 (their one-hot columns are zero).
        sl = idx_slots.reshape(c.NBLK, 2, LH)
        parts = []
        for b in range(c.NBLK):
            for h in range(2):
                parts.append(_pack_idx16(sl[b, h], c))
        idx_packed = np.concatenate(parts, axis=1)

        # dstoff: [p, b*2*NS + h*NS + s] = off_slots[b*2+h, s*128+p]
        dstoff = (
            off_slots.reshape(c.NBLK, 2, NS, 128)
            .transpose(3, 0, 1, 2)
            .reshape(128, c.NBLK * 2 * NS)
            .astype(ml_dtypes.bfloat16)
        )

        lpos = POS[core * c.NPC : (core + 1) * c.NPC] - core * c.NPAD
        dv = np.zeros(c.NPAD, np.float32)
        dv[lpos] = dinv[core * c.NPC : (core + 1) * c.NPC]
        dinvT = np.broadcast_to(dv, (c.H, c.NPAD))
        dinv_col = dv.reshape(c.NBLK, c.BLK).T.copy()  # [BLK, NBLK]

        ft = np.zeros((c.F, c.NPAD), np.float32)
        ft[:, lpos] = feat[core * c.NPC : (core + 1) * c.NPC].T
        per_core.append(
            dict(featT=ft, dinvT=dinvT, dinv_col=dinv_col, idx=idx_packed, dstoff=dstoff)
        )
    return per_core


def _build(cfg, LH):
    """Build the SPMD Bass graph (identical on all cores)."""
    c = cfg
    NS = -(-LH // 128)
    H = c.H
    NQ = 4                            # SWDGE queues

    nc = bacc.Bacc("TRN2", target_bir_lowering=False, debug=False,
                   num_devices=c.NCORES, num_swdge_queues=NQ,
                   dynamic_dma_scratch_size=32768)

    dram_in = {}

    def din(name, shape, dtype=F32):
        dram_in[name] = nc.dram_tensor(name, list(shape), dtype,
                                       kind="ExternalInput")
        return dram_in[name]

    for r in range(2):
        din(f"featT{r}", (c.F, c.NPAD))
        din(f"dinv_col{r}", (c.BLK, c.NBLK))
        din(f"idx{r}", (128, c.NBLK * 2 * (LH // 16)), I16)
        din(f"dstoff{r}", (128, c.NBLK * 2 * NS), BF16)
        din(f"W1_{r}", (c.F, H))
        din(f"W2_{r}", (H, H))
        din(f"b1_{r}", (H, 1))
        din(f"b2_{r}", (H, 1))
    din("dinvT", (128, c.NPAD))
    din("Vk", (H, 3 * H), BF16)
    din("ident_bf", (128, 128), BF16)
    din("b3x2", (H, 1))
    din("ident", (128, 128))
    din("iota", (128, 128), BF16)

    out_t = nc.dram_tensor("out", [c.NPAD, H], F32, kind="ExternalOutput")

    rg = [list(range(c.NCORES))]

    with tile.TileContext(nc) as tc:
        with (
            tc.tile_pool(name="const", bufs=1) as constp,
            tc.tile_pool(name="dram", bufs=1, space="DRAM") as dramp,
            tc.tile_pool(name="feat", bufs=3) as featp,
            tc.tile_pool(name="h1", bufs=2) as h1p,
            tc.tile_pool(name="idxp", bufs=8) as idxp,
            tc.tile_pool(name="oh", bufs=2) as ohp,
            tc.tile_pool(name="sc", bufs=4) as scp,
            tc.tile_pool(name="stg", bufs=3) as stgp,
            tc.tile_pool(name="psmlp", bufs=3, space="PSUM") as psmlp,
            tc.tile_pool(name="psagg", bufs=2, space="PSUM") as psagg,
            tc.tile_pool(name="psmisc", bufs=3, space="PSUM") as psmisc,
        ):
            # ---- constants to SBUF ----
            def load_const(name, shape, dtype=F32):
                t = constp.tile(list(shape), dtype, name=f"c_{name}")
                nc.sync.dma_start(out=t[:], in_=dram_in[name].ap()[:])
                return t

            W1 = [load_const(f"W1_{r}", (c.F, H)) for r in range(2)]
            W2 = [load_const(f"W2_{r}", (H, H)) for r in range(2)]
            b1 = [load_const(f"b1_{r}", (H, 1)) for r in range(2)]
            b2 = [load_const(f"b2_{r}", (H, 1)) for r in range(2)]
            dinvT2 = load_const("dinvT", (128, c.NPAD))
            dinv_col = [load_const(f"dinv_col{r}", (c.BLK, c.NBLK)) for r in range(2)]
            dstoff = [load_const(f"dstoff{r}", (128, c.NBLK * 2 * NS), BF16) for r in range(2)]
            Vk = load_const("Vk", (H, 3 * H), BF16)
            ident_bf = load_const("ident_bf", (128, 128), BF16)
            b3x2 = load_const("b3x2", (H, 1))
            ident = load_const("ident", (128, 128))
            iota = load_const("iota", (128, 128), BF16)

            # ---- persistent SBUF state ----
            P0T = [constp.tile([H, c.NPAD], BF16, name=f"P0T{r}") for r in range(2)]
            NSLOT = 16
            GTS = [constp.tile([128, NS, H], BF16, name=f"gtslot{i}")
                   for i in range(NSLOT)]
            for t in GTS:
                nc.gpsimd.memset(t[:], 0.0)
            P1T = [constp.tile([H, c.NPAD], BF16, name=f"P1T{r}") for r in range(2)]
            outaccT = constp.tile([H, c.NPAD], F32, name="outaccT")

            # ---- internal DRAM ----
            agin = [[dramp.tile([c.NPAD, 2 * H], BF16, name=f"agin{r}_{hp}")
                     for hp in range(2)] for r in range(2)]
            table = [[dramp.tile([c.NTAB, 2 * H], BF16, name=f"table{r}_{hp}")
                      for hp in range(2)] for r in range(2)]

            CHUNKS = []
            pos = 0
            while pos < c.NPAD:
                w = min(512, c.NPAD - pos)
                CHUNKS.append((pos, w))
                pos += w

            def write_scaled(PT, r, agin_t):
                """scaled rows (node-major) = dinv * P rows -> DRAM agin."""
                for b in range(c.NBLK):
                    bs = slice(b * c.BLK, (b + 1) * c.BLK)
                    tp = psmisc.tile([c.BLK, H], BF16, name="tp", tag="misc")
                    nc.tensor.transpose(tp[:], PT[:, bs], ident_bf[:H, :H])
                    stg = stgp.tile([c.BLK, H], BF16, name="stg")
                    nc.scalar.activation(stg[:], tp[:], AF.Copy,
                                         scale=dinv_col[r][:, b : b + 1])
                    nc.sync.dma_start(out=agin_t[bs, 0:H], in_=stg[:])

            def leaky(out_ap, in_ap, bias_ap, w):
                """out = lrelu(in + bias), via DVE (sim lacks ACT Lrelu)."""
                t = h1p.tile([H, 512], F32, name="lk", tag="lk")
                nc.vector.tensor_tensor(out=t[:, :w], in0=in_ap,
                                        in1=bias_ap.to_broadcast((H, w)),
                                        op=mybir.AluOpType.add)
                nc.vector.scalar_tensor_tensor(
                    out=out_ap, in0=t[:, :w], scalar=0.01, in1=t[:, :w],
                    op0=mybir.AluOpType.mult, op1=mybir.AluOpType.max)

            def mlp(r):
                for (p0, w) in CHUNKS:
                    ft = featp.tile([c.F, 512], F32, name="ft")
                    nc.sync.dma_start(out=ft[:, :w],
                                      in_=dram_in[f"featT{r}"].ap()[:, p0 : p0 + w])
                    ps1 = psmlp.tile([H, 512], F32, name="ps1", tag="mlp")
                    nc.tensor.matmul(ps1[:, :w], W1[r][:], ft[:, :w],
                                     start=True, stop=True)
                    h1t = h1p.tile([H, 512], F32, name="h1t")
                    leaky(h1t[:, :w], ps1[:, :w], b1[r][:], w)
                    ps2 = psmlp.tile([H, 512], F32, name="ps2", tag="mlp")
                    nc.tensor.matmul(ps2[:, :w], W2[r][:], h1t[:, :w],
                                     start=True, stop=True)
                    leaky(P0T[r][:, p0 : p0 + w], ps2[:, :w], b2[r][:], w)
                write_scaled(P0T[r], r, agin[r][0])

            def allgather(r, hp):
                nc.gpsimd.collective_compute(
                    "AllGather",
                    mybir.AluOpType.bypass,
                    replica_groups=rg,
                    ins=[agin[r][hp][:].opt()],
                    outs=[table[r][hp][:].opt()],
                )

            def prop(r, hop):
                """hop=1: P1T = L~ P0T (+ write scaled1). hop=2: fused output.

                Gathers stream over a flat subtile space per half in calls of
                CALL_SUB subtiles (<=1024 descriptors: the SWDGE carveout ring
                holds dynamic_dma_scratch_size/16 = 1024 descs; a single
                larger call deadlocks on HW). Calls rotate over 4 SWDGE
                queues.
                """
                tab = table[r][hop - 1]
                PTin = P0T[r] if hop == 1 else P1T[r]
                gts = [[None] * c.NBLK for _ in range(2)]
                M16 = LH // 16

                # split each (blk, half) gather into subtile-aligned pieces
                # small enough that several fit in a SWDGE ring (1024 descs),
                # so desc-gen pipelines with the drain instead of stalling.
                SPLIT = [(s0, min(s0 + 3, NS)) for s0 in range(0, NS, 3)]

                def issue_call(b):
                    for h in range(2):
                        it = idxp.tile([128, M16], I16, name="it")
                        ci = (b * 2 + h) * M16
                        nc.sync.dma_start(
                            out=it[:],
                            in_=dram_in[f"idx{r}"].ap()[:, ci : ci + M16])
                        gt = GTS[(b % (NSLOT // 2)) * 2 + h]
                        src_ap = (tab[0 : c.HALF, 0:H] if h == 0
                                  else tab[c.HALF :, 0:H])
                        for (s0, s1) in SPLIT:
                            n_i = min(LH - s0 * 128, (s1 - s0) * 128)
                            _dma_gather_narrow(
                                nc.gpsimd, gt[:, s0:s1, :], src_ap,
                                it[:, s0 * 8 : s0 * 8 + (n_i + 15) // 16],
                                n_i, n_i, H, 2 * H,
                                queue_num=(2 * b + h + s0) % NQ)
                        gts[h][b] = gt

                def do_block(b):
                    bs = slice(b * c.BLK, (b + 1) * c.BLK)
                    col0 = b * 2 * NS
                    oh = ohp.tile([128, 2 * NS, c.BLK], BF16, name="oh")
                    nc.vector.tensor_tensor(
                        out=oh[:],
                        in0=dstoff[r][:, col0 : col0 + 2 * NS][:, :, None]
                            .to_broadcast((128, 2 * NS, c.BLK)),
                        in1=iota[:][:, None, :c.BLK]
                            .to_broadcast((128, 2 * NS, c.BLK)),
                        op=mybir.AluOpType.is_equal,
                    )
                    agg = psagg.tile([H, c.BLK], F32, name="agg")
                    n_mm = 2 * NS
                    k = 0
                    for h in range(2):
                        for s in range(NS):
                            nc.tensor.matmul(
                                agg[:],
                                gts[h][b][:, s, :],
                                oh[:, h * NS + s, :],
                                start=(k == 0),
                                stop=(k == n_mm - 1),
                            )
                            k += 1
                    finish_block(b, bs, agg)

                def finish_block(b, bs, agg):
                    tmp = scp.tile([H, c.BLK], BF16, name="tmp")
                    nc.vector.tensor_tensor(out=tmp[:], in0=agg[:],
                                            in1=dinvT2[r * H : (r + 1) * H, bs],
                                            op=mybir.AluOpType.mult)
                    if hop == 1:
                        nc.vector.tensor_tensor(out=P1T[r][:, bs],
                                                in0=PTin[:, bs], in1=tmp[:],
                                                op=mybir.AluOpType.subtract)
                    else:
                        p2 = scp.tile([H, c.BLK], BF16, name="p2")
                        nc.vector.tensor_tensor(out=p2[:], in0=PTin[:, bs],
                                                in1=tmp[:],
                                                op=mybir.AluOpType.subtract)
                        op_ps = psmisc.tile([H, c.BLK], F32, name="opps", tag="misc")
                        nc.tensor.matmul(op_ps[:], Vk[:, 0:H], P0T[r][:, bs],
                                         start=True, stop=False)
                        nc.tensor.matmul(op_ps[:], Vk[:, H : 2 * H],
                                         P1T[r][:, bs], start=False, stop=False)
                        nc.tensor.matmul(op_ps[:], Vk[:, 2 * H : 3 * H], p2[:],
                                         start=False, stop=True)
                        if r == 0:
                            nc.vector.tensor_copy(out=outaccT[:, bs],
                                                  in_=op_ps[:])
                        else:
                            nc.vector.tensor_add(out=outaccT[:, bs],
                                                 in0=outaccT[:, bs],
                                                 in1=op_ps[:])

                LOOKAHEAD = 6
                for b in range(min(LOOKAHEAD, c.NBLK)):
                    issue_call(b)
                for b in range(c.NBLK):
                    if b + LOOKAHEAD < c.NBLK:
                        issue_call(b + LOOKAHEAD)
                    do_block(b)
                if hop == 1:
                    write_scaled(P1T[r], r, agin[r][1])

            def final():
                for b in range(c.NBLK):
                    bs = slice(b * c.BLK, (b + 1) * c.BLK)
                    lr = scp.tile([H, c.BLK], F32, name="lr")
                    leaky(lr[:], outaccT[:, bs], b3x2[:], c.BLK)
                    tp = psmisc.tile([c.BLK, H], F32, name="tpo", tag="misc")
                    nc.tensor.transpose(tp[:], lr[:], ident[:H, :H])
                    stg = stgp.tile([c.BLK, H], F32, name="stgo")
                    nc.vector.tensor_copy(out=stg[:], in_=tp[:])
                    nc.sync.dma_start(out=out_t.ap()[bs, :], in_=stg[:])

            mlp(0)
            allgather(0, 0)
            mlp(1)
            allgather(1, 0)
            prop(0, 1)
            allgather(0, 1)
            prop(1, 1)
            allgather(1, 1)
            prop(0, 2)
            prop(1, 2)
            final()

    nc.compile()
    return nc


def _prepare_with_cfg(inputs, cfg):
    r = _prepare(inputs, cfg)
    return r[0], r[1]


def _prepare(inputs, cfg):
    c = cfg
    W3 = inputs["W3"]
    H = c.H
    V = np.zeros((H, 3 * H), np.float32)
    for k in range(3):
        acc = np.zeros((H, H), np.float64)
        for i in range(3):
            acc += THETAS[i][k] * W3[i * H : (i + 1) * H].astype(np.float64)
        V[:, k * H : (k + 1) * H] = acc.astype(np.float32)

    srcs = [np.asarray(inputs["src_r1"]).astype(np.int64),
            np.asarray(inputs["src_r2"]).astype(np.int64)]
    dsts = [np.asarray(inputs["dst_r1"]).astype(np.int64),
            np.asarray(inputs["dst_r2"]).astype(np.int64)]

    # Node->block balancing permutation (shared by both relations): minimizes
    # the max per-(blk, rel, src-half) in-degree, i.e. the gather pad waste.
    POS = np.zeros(c.N, np.int64)
    for core in range(c.NCORES):
        cnt4 = np.zeros((c.NPC, 4), np.int64)
        for r in range(2):
            m = dsts[r] // c.NPC == core
            d_loc = dsts[r][m] - core * c.NPC
            half = (srcs[r][m] // (4 * c.NPC)).clip(0, 1)
            np.add.at(cnt4, (d_loc, 2 * r + half), 1)
        pos = _greedy_balance(cnt4, c)
        POS[core * c.NPC : (core + 1) * c.NPC] = core * c.NPAD + pos

    rels = []
    LH = 16
    for r, (fk, sk, dk) in enumerate(
        [("feat_r1", "src_r1", "dst_r1"), ("feat_r2", "src_r2", "dst_r2")]
    ):
        slots, dinv, lh = _prep_relation(
            np.asarray(inputs[fk]), srcs[r], dsts[r], c, POS)
        rels.append((slots, dinv, np.asarray(inputs[fk], np.float32)))
        LH = max(LH, lh)
    LH = ((LH + 15) // 16) * 16

    percore_r = []
    for r in range(2):
        slots, dinv, feat = rels[r]
        percore_r.append(_finalize_relation(slots, dinv, feat, c, LH, POS))

    ident = np.eye(128, dtype=np.float32)
    iota = np.broadcast_to(np.arange(128, dtype=np.float32), (128, 128)).copy()

    in_maps = []
    for core in range(c.NCORES):
        m = {}
        for r in range(2):
            pc = percore_r[r][core]
            m[f"featT{r}"] = pc["featT"]
            m[f"dinv_col{r}"] = pc["dinv_col"]
            m[f"idx{r}"] = pc["idx"]
            m[f"dstoff{r}"] = pc["dstoff"]
            suf = "_r1" if r == 0 else "_r2"
            m[f"W1_{r}"] = np.asarray(inputs[f"W1{suf}"], np.float32)
            m[f"W2_{r}"] = np.asarray(inputs[f"W2{suf}"], np.float32)
            m[f"b1_{r}"] = np.asarray(inputs[f"b1{suf}"], np.float32).reshape(H, 1)
            m[f"b2_{r}"] = np.asarray(inputs[f"b2{suf}"], np.float32).reshape(H, 1)
        m["dinvT"] = np.concatenate(
            [percore_r[0][core]["dinvT"], percore_r[1][core]["dinvT"]], axis=0
        ).copy()
        m["Vk"] = V.astype(ml_dtypes.bfloat16)
        m["b3x2"] = (2.0 * np.asarray(inputs["b3"], np.float32)).reshape(H, 1)
        m["ident"] = ident
        m["ident_bf"] = ident.astype(ml_dtypes.bfloat16)
        m["iota"] = iota.astype(ml_dtypes.bfloat16)
        in_maps.append(m)
    return in_maps, LH, POS


def _dma_gather_narrow(gp, out_ap, in_ap, idxs_ap, num_idxs, num_idxs_reg,
                       elem_size, elem_step, queue_num=0):
    """bass.BassGpSimd.dma_gather clone allowing elem_size_bytes % 256 != 0.

    The Q7 kernel (dma_gather.cpp gen_descs, non-transpose HBM path) supports
    any payload length; only the row STRIDE must encode as stride_bytes_256.
    Used to gather 128B bf16 rows from a 256B-strided table.
    """
    import concourse.ap_utils as ap_utils
    assert idxs_ap.dtype == I16
    assert in_ap.dtype == out_ap.dtype
    assert in_ap.space == bass.MemorySpace.DRAM
    assert idxs_ap.space == bass.MemorySpace.SBUF
    assert out_ap.space == bass.MemorySpace.SBUF
    assert ap_utils.ap_is_contiguous(out_ap.ap[1:])
    assert ap_utils.ap_is_contiguous(idxs_ap.ap[1:])
    assert in_ap.ap[0][0] == elem_step
    assert in_ap.ap[-1][1] == elem_size
    assert out_ap.ap[-1][1] == elem_size
    assert out_ap.ap[0][1] * out_ap.ap[1][1] * 1 >= num_idxs
    stride_bytes = elem_step * mybir.dt.size(in_ap.dtype)
    assert stride_bytes % 256 == 0 and stride_bytes // 256 < 256
    _in_ap = gp.lower_ap_dma(in_ap, for_custom_bir_dma=True)
    _idxs_ap = gp.lower_ap(idxs_ap)
    _out_ap = gp.lower_ap(out_ap)
    return gp.add_instruction(
        mybir.InstDMAGatherAnt(
            name=gp.bass.get_next_instruction_name(),
            ins=[*_in_ap, _idxs_ap, gp.lower_val_access(gp.to_reg(num_idxs_reg))],
            outs=[_out_ap],
            transpose=False,
            num_idxs=num_idxs,
            elem_size=elem_size,
            stride_bytes_256=stride_bytes // 256,
            gen_mode=0,
            single_packet=True,
            queue_num=queue_num,
            sbuf_tokens_per_rank=0,
            sbuf_free_dim_per_rank=0,
            sbuf_free_dim_pad_per_rank=0,
            sbuf_byte_offset=0,
        )
    )


_CACHE = {}


def _install_profile_shim():
    """Provide antenv.axon_hooks (missing in this image) so trace=True works."""
    try:
        from antenv.axon_hooks import get_axon_ntff_profile_hook  # noqa: F401
        return
    except ImportError:
        pass
    import types

    import antenv
    try:
        from trn_agent_boot.trn_boot import _ntff_profile_via_ctypes
        hook = _ntff_profile_via_ctypes("/opt/axon/libaxon_pjrt.so")
    except Exception:
        hook = None
    mod = types.ModuleType("antenv.axon_hooks")
    mod._hook = hook
    mod.get_axon_ntff_profile_hook = lambda: mod._hook

    def _set(h):
        mod._hook = h

    mod.set_axon_ntff_profile_hook = _set
    sys.modules["antenv.axon_hooks"] = mod
    antenv.axon_hooks = mod


def _run(inputs, trace=False, **kw):
    if trace:
        _install_profile_shim()
    cfg = Cfg(N=int(np.asarray(inputs["feat_r1"]).shape[0]))
    in_maps, LH, POS = _prepare(inputs, cfg)
    key = (cfg.N, LH)
    if key not in _CACHE:
        _CACHE[key] = _build(cfg, LH)
    nc = _CACHE[key]
    res = run_bass_kernel_spmd(nc, in_maps, core_ids=list(range(cfg.NCORES)),
                               trace=trace, **kw)
    outs = []
    for core in range(cfg.NCORES):
        lpos = POS[core * cfg.NPC : (core + 1) * cfg.NPC] - core * cfg.NPAD
        outs.append(np.asarray(res.results[core]["out"])[lpos])
    full = np.concatenate(outs, axis=0)
    return full, res


def kernel(**inputs):
    full, _ = _run(inputs, trace=False)
    return full

